# revision 25
# baseline (speedup 1.0000x reference)
"""DenseCoAttn Trainium2 kernel.

Full inputs -> shard batch (128) across 8 NeuronCores (16 batches/core) ->
Bass/Tile kernel per core -> gather.

Algorithm per batch (B=128, L1=196, L2=30, D1=2048, D2=1024, H=8, DK=128, NN=3):
  v1 = [none1; value1]  [199, 2048]     v2 = [none2; value2]  [33, 1024]
  q1 = v1 @ W1.T        [199, 1024]     q2 = v2 @ (W2/sqrt(DK)).T  [33, 1024]
  attn1: queries q2[3:33], keys q1, mask m1 -> mean over heads -> [30, 199]
  out1 = mean_attn1 @ v1                [30, 2048]
  attn2: queries q1[3:199], keys q2, mask m2 -> mean over heads -> [196, 33]
  out2 = mean_attn2 @ v2                [196, 1024]

Device-side restructurings:
  - head-mean commutes with the value matmul (V is shared across heads), so the
    per-head attention weights are combined BEFORE the V matmul (8x fewer flops).
  - masking is a rank-1 accumulating matmul (ones^T x mask_bias) on the PE.
  - attn1 scores stack 4 heads per PSUM bank on partitions (col-tiled matmuls);
    the per-head 1/sum and the head-mean are folded into a small "scatter"
    matrix S = E * recip(sums), applied as p^T @ S on the PE, which directly
    yields attn1^T as needed for the V matmul.
  - attn2 scores stack 8 heads along the free dim; head combine is a log-tree
    of DVE adds followed by one PE transpose.
  - all matmuls run in bf16 (fp32 accumulation in PSUM).
"""

import numpy as np
import ml_dtypes

B, L1, L2 = 128, 196, 30
D1, D2 = 2048, 1024
H, DK, NN = 8, 128, 3
L1P, L2P = L1 + NN, L2 + NN          # 199, 33
NCORES = 8
BC = B // NCORES                     # 16 batches per core
PAIRS = BC // 2                      # 8 pairs (2 batches share a proj matmul)
GB = 8                               # batches per q2-projection group

BF16 = ml_dtypes.bfloat16

_CACHE = {}


def _build_nc(pairs_limit=PAIRS):
    import concourse.bacc as bacc
    import concourse.tile as tile
    from concourse import mybir
    from contextlib import ExitStack

    bf16 = mybir.dt.bfloat16
    f32 = mybir.dt.float32
    X = mybir.AxisListType.X
    Exp = mybir.ActivationFunctionType.Exp

    nc = bacc.Bacc("TRN2", target_bir_lowering=False, debug=False)

    # DRAM I/O (per core)
    d_w1t = nc.dram_tensor("w1t", [D1, D2], bf16, kind="ExternalInput").ap()
    d_w2t = nc.dram_tensor("w2t", [D2, D2], bf16, kind="ExternalInput").ap()
    d_v1t = nc.dram_tensor("v1t", [PAIRS, D1, 2 * L1P], bf16, kind="ExternalInput").ap()
    d_v1n = nc.dram_tensor("v1n", [BC, L1P, D1], bf16, kind="ExternalInput").ap()
    d_v2tg = nc.dram_tensor("v2tg", [2, D2, GB * L2P], bf16, kind="ExternalInput").ap()
    d_v2n = nc.dram_tensor("v2n", [BC, L2P, D2], bf16, kind="ExternalInput").ap()
    d_mb1 = nc.dram_tensor("mb1", [1, BC * L1P], bf16, kind="ExternalInput").ap()
    d_mb2 = nc.dram_tensor("mb2", [1, BC * H * L2P], bf16, kind="ExternalInput").ap()
    d_ecst = nc.dram_tensor("ecst", [128, L2], bf16, kind="ExternalInput").ap()
    d_ones = nc.dram_tensor("onesv", [1, 128], bf16, kind="ExternalInput").ap()
    d_ident = nc.dram_tensor("ident", [128, 128], bf16, kind="ExternalInput").ap()
    d_o1 = nc.dram_tensor("o1", [BC, L2, D1], f32, kind="ExternalOutput").ap()
    d_o2 = nc.dram_tensor("o2", [BC, L1, D2], f32, kind="ExternalOutput").ap()

    with tile.TileContext(nc) as tc, ExitStack() as ctx:
        wpool = ctx.enter_context(tc.tile_pool(name="wpool", bufs=1))
        cpool = ctx.enter_context(tc.tile_pool(name="cpool", bufs=1))
        rpool = ctx.enter_context(tc.tile_pool(name="rpool", bufs=1))
        v1pool = ctx.enter_context(tc.tile_pool(name="v1pool", bufs=2))
        qpool = ctx.enter_context(tc.tile_pool(name="qpool", bufs=3))
        wk = ctx.enter_context(tc.tile_pool(name="wk", bufs=2))
        opool = ctx.enter_context(tc.tile_pool(name="opool", bufs=4))
        projp = ctx.enter_context(tc.tile_pool(name="projp", bufs=2, space="PSUM"))
        scp1 = ctx.enter_context(tc.tile_pool(name="scp1", bufs=2, space="PSUM"))
        scp2 = ctx.enter_context(tc.tile_pool(name="scp2", bufs=1, space="PSUM"))
        smallp = ctx.enter_context(tc.tile_pool(name="smallp", bufs=1, space="PSUM"))
        outp = ctx.enter_context(tc.tile_pool(name="outp", bufs=2, space="PSUM"))

        # ---- q2 projection first (shortest DMA critical path to first MM) ----
        w2sb = []
        for i in range(D2 // 128):
            t = wpool.tile([128, D2], bf16, tag=f"w2_{i}")
            nc.sync.dma_start(t[:], d_w2t[128 * i:128 * (i + 1), :])
            w2sb.append(t)

        q2sb = [[None] * H for _ in range(2)]

        def emit_q2_dma(g):
            v2t = []
            for d in range(D2 // 128):
                t = v1pool.tile([128, GB * L2P], bf16, tag=f"v1t_{d}", bufs=3)
                nc.sync.dma_start(t[:], d_v2tg[g, 128 * d:128 * (d + 1), :])
                v2t.append(t)
            return v2t

        def emit_q2_proj(g, v2t):
            for h in range(H):
                ps = projp.tile([128, 512], f32, tag="proj")
                for d in range(D2 // 128):
                    nc.tensor.matmul(ps[:, 0:GB * L2P], w2sb[d][:, 128 * h:128 * (h + 1)],
                                     v2t[d][:], start=(d == 0), stop=(d == D2 // 128 - 1))
                q = rpool.tile([128, GB * L2P], bf16, tag=f"q2t_{g}_{h}")
                nc.scalar.copy(q[:], ps[:, 0:GB * L2P])
                q2sb[g][h] = q

        v2t0 = emit_q2_dma(0)

        # w1t streams during the q2 projection compute
        w1sb = []
        for i in range(D1 // 128):
            t = wpool.tile([128, D2], bf16, tag=f"w1_{i}")
            nc.sync.dma_start(t[:], d_w1t[128 * i:128 * (i + 1), :])
            w1sb.append(t)

        emit_q2_proj(0, v2t0)
        ecst = cpool.tile([128, L2], bf16, tag="ecst")
        nc.sync.dma_start(ecst[:], d_ecst[:])
        ones = cpool.tile([1, 128], bf16, tag="ones")
        nc.sync.dma_start(ones[:], d_ones[:])
        ident = cpool.tile([128, 128], bf16, tag="ident")
        nc.sync.dma_start(ident[:], d_ident[:])
        mb1 = cpool.tile([1, BC * L1P], bf16, tag="mb1")
        nc.sync.dma_start(mb1[:], d_mb1[:])
        mb2 = cpool.tile([1, BC * H * L2P], bf16, tag="mb2")
        nc.sync.dma_start(mb2[:], d_mb2[:])


        # ---- main loop over pairs of batches ----
        for p in range(pairs_limit):
            if p == PAIRS // 2 and pairs_limit == PAIRS:
                # group-1 q2 projection just in time (PE filler mid-kernel)
                emit_q2_proj(1, emit_q2_dma(1))
            v1t = []
            for d in range(D1 // 128):
                t = v1pool.tile([128, 2 * L1P], bf16, tag=f"v1t_{d}", bufs=3)
                nc.sync.dma_start(t[:], d_v1t[p, 128 * d:128 * (d + 1), :])
                v1t.append(t)
            # q1 projection for both batches of the pair
            q1sb = []
            for h in range(H):
                ps = projp.tile([128, 512], f32, tag="proj")
                for d in range(D1 // 128):
                    nc.tensor.matmul(ps[:, 0:2 * L1P], w1sb[d][:, 128 * h:128 * (h + 1)],
                                     v1t[d][:], start=(d == 0), stop=(d == D1 // 128 - 1))
                q = qpool.tile([128, 2 * L1P], bf16, tag=f"q1t_{h}")
                nc.scalar.copy(q[:], ps[:, 0:2 * L1P])
                q1sb.append(q)

            for j in range(2):
                b = 2 * p + j
                g, bg = b // GB, b % GB
                q1o = L1P * j
                v1na = v1pool.tile([128, D1], bf16, tag=f"v1na_{j}")
                nc.sync.dma_start(v1na[:], d_v1n[b, 0:128, :])
                v1nb = v1pool.tile([L1P - 128, D1], bf16, tag=f"v1nb_{j}")
                nc.sync.dma_start(v1nb[:], d_v1n[b, 128:L1P, :])
                v2nb = v1pool.tile([L2P, D2], bf16, tag=f"v2n_{j}")
                nc.sync.dma_start(v2nb[:], d_v2n[b])

                # ======== attention 1 ========
                psb, ssb = [], []
                for half in range(2):
                    sc = scp1.tile([128, 512], f32, tag="sc1")
                    for t4 in range(4):
                        h = 4 * half + t4
                        # queries 1..32 (M=32 fills the whole col-group; the
                        # wanted queries 3..32 sit at rows 32*t4 + 2..31)
                        nc.tensor.matmul(
                            sc[32 * t4:32 * t4 + 32, 0:L1P],
                            q2sb[g][h][:, L2P * bg + 1:L2P * bg + L2P],
                            q1sb[h][:, q1o:q1o + L1P],
                            start=True, stop=False,
                            tile_position=(0, 32 * t4),
                            skip_group_check=True)
                    nc.tensor.matmul(sc[:, 0:L1P], ones[0:1, 0:128],
                                     mb1[0:1, L1P * b:L1P * (b + 1)],
                                     start=False, stop=True,
                                     skip_group_check=True)
                    negmax = wk.tile([128, 1], f32, tag="negmax1")
                    nc.vector.reduce_max(negmax[:], sc[:, 0:L1P], axis=X, negate=True)
                    pt = wk.tile([128, L1P], bf16, tag=f"p1_{half}")
                    sums = wk.tile([128, 1], f32, tag="sums1")
                    nc.scalar.activation(pt[:], sc[:, 0:L1P], Exp,
                                         bias=negmax[:], accum_out=sums[:])
                    r = wk.tile([128, 1], f32, tag="r1")
                    nc.vector.reciprocal(r[:], sums[:])
                    st = wk.tile([128, L2], bf16, tag=f"s1_{half}")
                    nc.vector.tensor_scalar_mul(st[:], ecst[:], r[:])
                    psb.append(pt)
                    ssb.append(st)
                # attn1^T = p_stacked^T @ S  (both halves accumulated)
                a1ps = smallp.tile([128, 512], f32, tag="small")
                nc.tensor.matmul(a1ps[0:128, 0:L2], psb[0][:, 0:128],
                                 ssb[0][:], start=True, stop=False)
                nc.tensor.matmul(a1ps[0:128, 0:L2], psb[1][:, 0:128],
                                 ssb[1][:], start=False, stop=True)
                nc.tensor.matmul(a1ps[0:L1P - 128, L2:2 * L2], psb[0][:, 128:L1P],
                                 ssb[0][:], start=True, stop=False)
                nc.tensor.matmul(a1ps[0:L1P - 128, L2:2 * L2], psb[1][:, 128:L1P],
                                 ssb[1][:], start=False, stop=True)
                a1ta = wk.tile([128, L2], bf16, tag="a1ta")
                a1tb = wk.tile([L1P - 128, L2], bf16, tag="a1tb")
                nc.scalar.copy(a1ta[:], a1ps[0:128, 0:L2])
                nc.scalar.copy(a1tb[:], a1ps[0:L1P - 128, L2:2 * L2])
                # out1 = attn1 @ v1
                for nh in range(4):
                    o1ps = outp.tile([L2, 512], f32, tag="outps")
                    n0 = 512 * nh
                    nc.tensor.matmul(o1ps[:], a1ta[:], v1na[:, n0:n0 + 512],
                                     start=True, stop=False)
                    nc.tensor.matmul(o1ps[:], a1tb[:], v1nb[:, n0:n0 + 512],
                                     start=False, stop=True)
                    o1sb = opool.tile([L2, 512], f32, tag="o1sb")
                    nc.scalar.copy(o1sb[:], o1ps[:])
                    nc.sync.dma_start(d_o1[b, :, n0:n0 + 512], o1sb[:])

                # ======== attention 2 ========
                a2t = wk.tile([L2P, L1], bf16, tag="a2t")
                for c, (qb, qn) in enumerate([(NN, 128), (NN + 128, L1 - 128)]):
                    sc2 = scp2.tile([128, 512], f32, tag="sc2")
                    for h in range(H):
                        nc.tensor.matmul(
                            sc2[0:qn, L2P * h:L2P * (h + 1)],
                            q1sb[h][:, q1o + qb:q1o + qb + qn],
                            q2sb[g][h][:, L2P * bg:L2P * (bg + 1)],
                            start=(h == 0), stop=False)
                    nc.tensor.matmul(sc2[0:qn, :], ones[0:1, 0:qn],
                                     mb2[0:1, H * L2P * b:H * L2P * (b + 1)],
                                     start=False, stop=True)
                    negmax2 = wk.tile([128, H], f32, tag="negmax2")
                    nc.vector.reduce_max(
                        negmax2[0:qn, :],
                        sc2[0:qn, :].rearrange("p (h k) -> p h k", h=H),
                        axis=X, negate=True)
                    p2pre = wk.tile([128, H * L2P], bf16, tag="p2pre")
                    nc.vector.tensor_add(
                        p2pre[0:qn, :].rearrange("p (h k) -> p h k", h=H),
                        sc2[0:qn, :].rearrange("p (h k) -> p h k", h=H),
                        negmax2[0:qn, :].broadcast_to((qn, H, L2P)))
                    p2 = wk.tile([128, H * L2P], bf16, tag="p2")
                    nc.scalar.activation(p2[0:qn, :], p2pre[0:qn, :], Exp)
                    sums2 = wk.tile([128, H], f32, tag="sums2")
                    nc.vector.reduce_sum(
                        sums2[0:qn, :],
                        p2[0:qn, :].rearrange("p (h k) -> p h k", h=H),
                        axis=X)
                    r2 = wk.tile([128, H], f32, tag="r2")
                    nc.vector.reciprocal(r2[0:qn, :], sums2[0:qn, :])
                    p2w = wk.tile([128, H * L2P], bf16, tag="p2w")
                    nc.vector.tensor_mul(
                        p2w[0:qn, :].rearrange("p (h k) -> p h k", h=H),
                        p2[0:qn, :].rearrange("p (h k) -> p h k", h=H),
                        r2[0:qn, :].broadcast_to((qn, H, L2P)))
                    t1 = wk.tile([128, 4 * L2P], bf16, tag="t1")
                    nc.vector.tensor_add(t1[0:qn, :], p2w[0:qn, 0:4 * L2P],
                                         p2w[0:qn, 4 * L2P:8 * L2P])
                    t2 = wk.tile([128, 2 * L2P], bf16, tag="t2")
                    nc.vector.tensor_add(t2[0:qn, :], t1[0:qn, 0:2 * L2P],
                                         t1[0:qn, 2 * L2P:4 * L2P])
                    a2s = wk.tile([128, L2P], bf16, tag="a2s")
                    nc.vector.tensor_add(a2s[0:qn, :], t2[0:qn, 0:L2P],
                                         t2[0:qn, L2P:2 * L2P])
                    a2ps = smallp.tile([L2P, 1024], bf16, tag="small")
                    nc.tensor.transpose(a2ps[0:L2P, 0:qn], a2s[0:qn, 0:L2P],
                                        ident[0:qn, 0:qn])
                    nc.scalar.copy(a2t[:, 128 * c:128 * c + qn], a2ps[0:L2P, 0:qn])
                    # out2 chunk immediately (overlaps the other chunk's softmax)
                    orow = 0 if c == 0 else 128
                    for nn2 in range(2):
                        o2ps = outp.tile([128, 512], f32, tag="outps")
                        nc.tensor.matmul(o2ps[0:qn, :],
                                         a2t[:, 128 * c:128 * c + qn],
                                         v2nb[:, 512 * nn2:512 * (nn2 + 1)],
                                         start=True, stop=True)
                        o2sb = opool.tile([128, 512], f32, tag="o2sb")
                        nc.vector.tensor_copy(o2sb[0:qn, :], o2ps[0:qn, :])
                        nc.sync.dma_start(d_o2[b, orow:orow + qn, 512 * nn2:512 * (nn2 + 1)],
                                          o2sb[0:qn, :])

    nc.compile()
    return nc


def _host_prep(value1, value2, mask1, mask2, W1, W2, none1, none2):
    """Build per-core input maps (numpy only)."""
    value1 = np.asarray(value1, dtype=np.float32)
    value2 = np.asarray(value2, dtype=np.float32)
    mask1 = np.asarray(mask1, dtype=np.float32)
    mask2 = np.asarray(mask2, dtype=np.float32)
    W1 = np.asarray(W1, dtype=np.float32)
    W2 = np.asarray(W2, dtype=np.float32)
    none1 = np.asarray(none1, dtype=np.float32)
    none2 = np.asarray(none2, dtype=np.float32)

    w1t = np.ascontiguousarray(W1.T).astype(BF16)                      # [2048, 1024]
    w2t = np.ascontiguousarray((W2 / np.sqrt(DK)).T).astype(BF16)      # [1024, 1024]
    ecst = np.zeros((128, L2), np.float32)
    for hh in range(4):
        # query q (3..32) sits at row 32*hh + (q - 1); output col is q - 3
        ecst[32 * hh + np.arange(L2) + 2, np.arange(L2)] = 1.0 / H
    ecst = ecst.astype(BF16)
    onesv = np.ones((1, 128), BF16)
    ident = np.eye(128, dtype=np.float32).astype(BF16)

    shared = {"w1t": w1t, "w2t": w2t, "ecst": ecst, "onesv": onesv, "ident": ident}

    in_maps = []
    for c in range(NCORES):
        sl = slice(BC * c, BC * (c + 1))
        v1 = np.concatenate(
            [np.broadcast_to(none1[None], (BC, NN, D1)), value1[sl]], axis=1)  # [16,199,2048]
        v2 = np.concatenate(
            [np.broadcast_to(none2[None], (BC, NN, D2)), value2[sl]], axis=1)  # [16,33,1024]
        v1n = v1.astype(BF16)
        v1t = (v1.transpose(0, 2, 1)                                   # [16,2048,199]
               .reshape(PAIRS, 2, D1, L1P).transpose(0, 2, 1, 3)
               .reshape(PAIRS, D1, 2 * L1P)).astype(BF16)
        v2n = (v2 / H).astype(BF16)
        v2tg = (v2.transpose(0, 2, 1)                                  # [16,1024,33]
                .reshape(2, GB, D2, L2P).transpose(0, 2, 1, 3)
                .reshape(2, D2, GB * L2P)).astype(BF16)
        m1f = np.concatenate([np.ones((BC, NN), np.float32), mask1[sl]], axis=1)
        m2f = np.concatenate([np.ones((BC, NN), np.float32), mask2[sl]], axis=1)
        mb1 = ((m1f - 1.0) * 30000.0).astype(BF16).reshape(1, BC * L1P)
        mb2 = np.tile(((m2f - 1.0) * 30000.0), (1, H)).astype(BF16).reshape(1, BC * H * L2P)
        in_maps.append(dict(shared, v1t=np.ascontiguousarray(v1t),
                            v1n=np.ascontiguousarray(v1n),
                            v2tg=np.ascontiguousarray(v2tg),
                            v2n=np.ascontiguousarray(v2n),
                            mb1=mb1, mb2=mb2))
    return in_maps


def _build_callable(nc):
    """jit-once PJRT callable over 8 cores (mirrors bass2jax.run_bass_via_pjrt)."""
    import jax
    from jax.sharding import Mesh, PartitionSpec
    from jax.experimental.shard_map import shard_map
    from concourse import bass2jax, mybir

    bass2jax.install_neuronx_cc_hook()
    partition_name = nc.partition_id_tensor.name if nc.partition_id_tensor else None
    in_names, out_names, out_avals, zero_outs = [], [], [], []
    for alloc in nc.m.functions[0].allocations:
        if not isinstance(alloc, mybir.MemoryLocationSet):
            continue
        name = alloc.memorylocations[0].name
        if alloc.kind == "ExternalInput":
            if name != partition_name:
                in_names.append(name)
        elif alloc.kind == "ExternalOutput":
            out_names.append(name)
            shape = tuple(alloc.tensor_shape)
            dtype = mybir.dt.np(alloc.dtype)
            out_avals.append(jax.core.ShapedArray(shape, dtype))
            zero_outs.append(np.zeros(shape, dtype))
    all_in_names = list(in_names) + list(out_names)
    if partition_name is not None:
        all_in_names.append(partition_name)

    def _body(*args):
        operands = list(args)
        if partition_name is not None:
            operands.append(bass2jax.partition_id_tensor())
        outs = bass2jax._bass_exec_p.bind(
            *operands,
            out_avals=tuple(out_avals),
            in_names=tuple(all_in_names),
            out_names=tuple(out_names),
            lowering_input_output_aliases=(),
            sim_require_finite=True,
            sim_require_nnan=True,
            nc=nc,
        )
        return tuple(outs)

    devices = jax.devices()[:NCORES]
    mesh = Mesh(np.asarray(devices), ("core",))
    n = len(in_names) + len(out_avals)
    fn = jax.jit(
        shard_map(_body, mesh=mesh, in_specs=(PartitionSpec("core"),) * n,
                  out_specs=(PartitionSpec("core"),) * len(out_names),
                  check_rep=False),
        keep_unused=True)
    return fn, in_names, out_names, zero_outs


def kernel(value1, value2, mask1, mask2, W1, W2, none1, none2):
    in_maps = _host_prep(value1, value2, mask1, mask2, W1, W2, none1, none2)

    if "nc" not in _CACHE:
        _CACHE["nc"] = _build_nc()
    nc = _CACHE["nc"]

    try:
        if "fn" not in _CACHE:
            _CACHE["fn"] = _build_callable(nc)
        fn, in_names, out_names, zero_outs = _CACHE["fn"]
        cat_in = [np.concatenate([np.asarray(m[name]) for m in in_maps], axis=0)
                  for name in in_names]
        cat_zero = [np.zeros((NCORES * z.shape[0], *z.shape[1:]), z.dtype)
                    for z in zero_outs]
        outs = fn(*cat_in, *cat_zero)
        o1 = np.asarray(outs[out_names.index("o1")])
        o2 = np.asarray(outs[out_names.index("o2")])
        return (np.ascontiguousarray(o1.reshape(B, L2, D1).astype(np.float32)),
                np.ascontiguousarray(o2.reshape(B, L1, D2).astype(np.float32)))
    except Exception:
        from concourse.bass_utils import run_bass_kernel_spmd
        res = run_bass_kernel_spmd(nc, in_maps, core_ids=list(range(NCORES)))
        out1 = np.empty((B, L2, D1), np.float32)
        out2 = np.empty((B, L1, D2), np.float32)
        for c in range(NCORES):
            out1[BC * c:BC * (c + 1)] = res.results[c]["o1"]
            out2[BC * c:BC * (c + 1)] = res.results[c]["o2"]
        return out1, out2


# revision 27
# speedup vs baseline: 1.0003x; 1.0003x over previous
"""DenseCoAttn Trainium2 kernel.

Full inputs -> shard batch (128) across 8 NeuronCores (16 batches/core) ->
Bass/Tile kernel per core -> gather.

Algorithm per batch (B=128, L1=196, L2=30, D1=2048, D2=1024, H=8, DK=128, NN=3):
  v1 = [none1; value1]  [199, 2048]     v2 = [none2; value2]  [33, 1024]
  q1 = v1 @ W1.T        [199, 1024]     q2 = v2 @ (W2/sqrt(DK)).T  [33, 1024]
  attn1: queries q2[3:33], keys q1, mask m1 -> mean over heads -> [30, 199]
  out1 = mean_attn1 @ v1                [30, 2048]
  attn2: queries q1[3:199], keys q2, mask m2 -> mean over heads -> [196, 33]
  out2 = mean_attn2 @ v2                [196, 1024]

Device-side restructurings:
  - head-mean commutes with the value matmul (V is shared across heads), so the
    per-head attention weights are combined BEFORE the V matmul (8x fewer flops).
  - masking is a rank-1 accumulating matmul (ones^T x mask_bias) on the PE.
  - attn1 scores stack 4 heads per PSUM bank on partitions (col-tiled matmuls);
    the per-head 1/sum and the head-mean are folded into a small "scatter"
    matrix S = E * recip(sums), applied as p^T @ S on the PE, which directly
    yields attn1^T as needed for the V matmul.
  - attn2 scores stack 8 heads along the free dim; head combine is a log-tree
    of DVE adds followed by one PE transpose.
  - all matmuls run in bf16 (fp32 accumulation in PSUM).
"""

import numpy as np
import ml_dtypes

B, L1, L2 = 128, 196, 30
D1, D2 = 2048, 1024
H, DK, NN = 8, 128, 3
L1P, L2P = L1 + NN, L2 + NN          # 199, 33
NCORES = 8
BC = B // NCORES                     # 16 batches per core
PAIRS = BC // 2                      # 8 pairs (2 batches share a proj matmul)
GB = 8                               # batches per q2-projection group

BF16 = ml_dtypes.bfloat16

_CACHE = {}


def _build_nc(pairs_limit=PAIRS):
    import concourse.bacc as bacc
    import concourse.tile as tile
    from concourse import mybir
    from contextlib import ExitStack

    bf16 = mybir.dt.bfloat16
    f32 = mybir.dt.float32
    X = mybir.AxisListType.X
    Exp = mybir.ActivationFunctionType.Exp

    nc = bacc.Bacc("TRN2", target_bir_lowering=False, debug=False)

    # DRAM I/O (per core)
    d_w1t = nc.dram_tensor("w1t", [H, 128, D1], bf16, kind="ExternalInput").ap()
    d_w2t = nc.dram_tensor("w2t", [H, 128, D2], bf16, kind="ExternalInput").ap()
    d_v1t = nc.dram_tensor("v1t", [PAIRS, D1, 2 * L1P], bf16, kind="ExternalInput").ap()
    d_v1n = nc.dram_tensor("v1n", [BC, L1P, D1], bf16, kind="ExternalInput").ap()
    d_v2tg = nc.dram_tensor("v2tg", [2, D2, GB * L2P], bf16, kind="ExternalInput").ap()
    d_v2n = nc.dram_tensor("v2n", [BC, L2P, D2], bf16, kind="ExternalInput").ap()
    d_mb1 = nc.dram_tensor("mb1", [1, BC * L1P], bf16, kind="ExternalInput").ap()
    d_mb2 = nc.dram_tensor("mb2", [1, BC * H * L2P], bf16, kind="ExternalInput").ap()
    d_ecst = nc.dram_tensor("ecst", [128, L2], bf16, kind="ExternalInput").ap()
    d_ones = nc.dram_tensor("onesv", [1, 128], bf16, kind="ExternalInput").ap()
    d_ident = nc.dram_tensor("ident", [128, 128], bf16, kind="ExternalInput").ap()
    d_o1 = nc.dram_tensor("o1", [BC, L2, D1], f32, kind="ExternalOutput").ap()
    d_o2 = nc.dram_tensor("o2", [BC, L1, D2], f32, kind="ExternalOutput").ap()

    with tile.TileContext(nc) as tc, ExitStack() as ctx:
        wpool = ctx.enter_context(tc.tile_pool(name="wpool", bufs=1))
        cpool = ctx.enter_context(tc.tile_pool(name="cpool", bufs=1))
        rpool = ctx.enter_context(tc.tile_pool(name="rpool", bufs=1))
        v1pool = ctx.enter_context(tc.tile_pool(name="v1pool", bufs=2))
        qpool = ctx.enter_context(tc.tile_pool(name="qpool", bufs=3))
        wk = ctx.enter_context(tc.tile_pool(name="wk", bufs=2))
        opool = ctx.enter_context(tc.tile_pool(name="opool", bufs=4))
        projp = ctx.enter_context(tc.tile_pool(name="projp", bufs=2, space="PSUM"))
        scp1 = ctx.enter_context(tc.tile_pool(name="scp1", bufs=2, space="PSUM"))
        scp2 = ctx.enter_context(tc.tile_pool(name="scp2", bufs=1, space="PSUM"))
        smallp = ctx.enter_context(tc.tile_pool(name="smallp", bufs=1, space="PSUM"))
        outp = ctx.enter_context(tc.tile_pool(name="outp", bufs=2, space="PSUM"))

        # ---- q2 projection first (shortest DMA critical path to first MM) ----
        q2sb = [[None] * H for _ in range(2)]
        w2sb = []
        for i in range(H):
            t = wpool.tile([128, D2], bf16, tag=f"w2_{i}")
            nc.sync.dma_start(t[:], d_w2t[i])
            w2sb.append(t)

        def emit_q2_dma(g):
            v2t = []
            for d in range(D2 // 128):
                t = v1pool.tile([128, GB * L2P], bf16, tag=f"v1t_{d}", bufs=3)
                nc.sync.dma_start(t[:], d_v2tg[g, 128 * d:128 * (d + 1), :])
                v2t.append(t)
            return v2t

        def emit_q2_proj(g, v2t):
            for h in range(H):
                ps = projp.tile([128, 512], f32, tag="proj")
                for d in range(D2 // 128):
                    nc.tensor.matmul(ps[:, 0:GB * L2P], w2sb[h][:, 128 * d:128 * (d + 1)],
                                     v2t[d][:], start=(d == 0), stop=(d == D2 // 128 - 1))
                q = rpool.tile([128, GB * L2P], bf16, tag=f"q2t_{g}_{h}")
                nc.scalar.copy(q[:], ps[:, 0:GB * L2P])
                q2sb[g][h] = q

        v2t0 = emit_q2_dma(0)

        # w1t streams during the q2 projection compute
        w1sb = []
        for i in range(H):
            t = wpool.tile([128, D1], bf16, tag=f"w1_{i}")
            nc.sync.dma_start(t[:], d_w1t[i])
            w1sb.append(t)

        emit_q2_proj(0, v2t0)
        ecst = cpool.tile([128, L2], bf16, tag="ecst")
        nc.sync.dma_start(ecst[:], d_ecst[:])
        ones = cpool.tile([1, 128], bf16, tag="ones")
        nc.sync.dma_start(ones[:], d_ones[:])
        ident = cpool.tile([128, 128], bf16, tag="ident")
        nc.sync.dma_start(ident[:], d_ident[:])
        mb1 = cpool.tile([1, BC * L1P], bf16, tag="mb1")
        nc.sync.dma_start(mb1[:], d_mb1[:])
        mb2 = cpool.tile([1, BC * H * L2P], bf16, tag="mb2")
        nc.sync.dma_start(mb2[:], d_mb2[:])


        # ---- main loop over pairs of batches ----
        for p in range(pairs_limit):
            if p == PAIRS // 2 and pairs_limit == PAIRS:
                # group-1 q2 projection just in time (PE filler mid-kernel)
                emit_q2_proj(1, emit_q2_dma(1))
            v1t = []
            for d in range(D1 // 128):
                t = v1pool.tile([128, 2 * L1P], bf16, tag=f"v1t_{d}", bufs=3)
                nc.sync.dma_start(t[:], d_v1t[p, 128 * d:128 * (d + 1), :])
                v1t.append(t)
            # q1 projection for both batches of the pair
            q1sb = []
            for h in range(H):
                ps = projp.tile([128, 512], f32, tag="proj")
                for d in range(D1 // 128):
                    nc.tensor.matmul(ps[:, 0:2 * L1P], w1sb[h][:, 128 * d:128 * (d + 1)],
                                     v1t[d][:], start=(d == 0), stop=(d == D1 // 128 - 1))
                q = qpool.tile([128, 2 * L1P], bf16, tag=f"q1t_{h}")
                nc.scalar.copy(q[:], ps[:, 0:2 * L1P])
                q1sb.append(q)

            for j in range(2):
                b = 2 * p + j
                g, bg = b // GB, b % GB
                q1o = L1P * j
                v1na = v1pool.tile([128, D1], bf16, tag=f"v1na_{j}")
                nc.sync.dma_start(v1na[:], d_v1n[b, 0:128, :])
                v1nb = v1pool.tile([L1P - 128, D1], bf16, tag=f"v1nb_{j}")
                nc.sync.dma_start(v1nb[:], d_v1n[b, 128:L1P, :])
                v2nb = v1pool.tile([L2P, D2], bf16, tag=f"v2n_{j}")
                nc.sync.dma_start(v2nb[:], d_v2n[b])

                # ======== attention 1 ========
                psb, ssb = [], []
                for half in range(2):
                    sc = scp1.tile([128, 512], f32, tag="sc1")
                    for t4 in range(4):
                        h = 4 * half + t4
                        # queries 1..32 (M=32 fills the whole col-group; the
                        # wanted queries 3..32 sit at rows 32*t4 + 2..31)
                        nc.tensor.matmul(
                            sc[32 * t4:32 * t4 + 32, 0:L1P],
                            q2sb[g][h][:, L2P * bg + 1:L2P * bg + L2P],
                            q1sb[h][:, q1o:q1o + L1P],
                            start=True, stop=False,
                            tile_position=(0, 32 * t4),
                            skip_group_check=True)
                    nc.tensor.matmul(sc[:, 0:L1P], ones[0:1, 0:128],
                                     mb1[0:1, L1P * b:L1P * (b + 1)],
                                     start=False, stop=True,
                                     skip_group_check=True)
                    pt = wk.tile([128, L1P], bf16, tag=f"p1_{half}")
                    sums = wk.tile([128, 1], f32, tag="sums1")
                    nc.scalar.activation(pt[:], sc[:, 0:L1P], Exp,
                                         accum_out=sums[:])
                    r = wk.tile([128, 1], f32, tag="r1")
                    nc.vector.reciprocal(r[:], sums[:])
                    st = wk.tile([128, L2], bf16, tag=f"s1_{half}")
                    nc.vector.tensor_scalar_mul(st[:], ecst[:], r[:])
                    psb.append(pt)
                    ssb.append(st)
                # attn1^T = p_stacked^T @ S  (both halves accumulated)
                a1ps = smallp.tile([128, 512], f32, tag="small")
                nc.tensor.matmul(a1ps[0:128, 0:L2], psb[0][:, 0:128],
                                 ssb[0][:], start=True, stop=False)
                nc.tensor.matmul(a1ps[0:128, 0:L2], psb[1][:, 0:128],
                                 ssb[1][:], start=False, stop=True)
                nc.tensor.matmul(a1ps[0:L1P - 128, L2:2 * L2], psb[0][:, 128:L1P],
                                 ssb[0][:], start=True, stop=False)
                nc.tensor.matmul(a1ps[0:L1P - 128, L2:2 * L2], psb[1][:, 128:L1P],
                                 ssb[1][:], start=False, stop=True)
                a1ta = wk.tile([128, L2], bf16, tag="a1ta")
                a1tb = wk.tile([L1P - 128, L2], bf16, tag="a1tb")
                nc.scalar.copy(a1ta[:], a1ps[0:128, 0:L2])
                nc.scalar.copy(a1tb[:], a1ps[0:L1P - 128, L2:2 * L2])
                # out1 = attn1 @ v1
                for nh in range(4):
                    o1ps = outp.tile([L2, 512], f32, tag="outps")
                    n0 = 512 * nh
                    nc.tensor.matmul(o1ps[:], a1ta[:], v1na[:, n0:n0 + 512],
                                     start=True, stop=False)
                    nc.tensor.matmul(o1ps[:], a1tb[:], v1nb[:, n0:n0 + 512],
                                     start=False, stop=True)
                    o1sb = opool.tile([L2, 512], f32, tag="o1sb")
                    nc.scalar.copy(o1sb[:], o1ps[:])
                    nc.sync.dma_start(d_o1[b, :, n0:n0 + 512], o1sb[:])

                # ======== attention 2 ========
                a2t = wk.tile([L2P, L1], bf16, tag="a2t")
                for c, (qb, qn) in enumerate([(NN, 128), (NN + 128, L1 - 128)]):
                    sc2 = scp2.tile([128, 512], f32, tag="sc2")
                    for h in range(H):
                        nc.tensor.matmul(
                            sc2[0:qn, L2P * h:L2P * (h + 1)],
                            q1sb[h][:, q1o + qb:q1o + qb + qn],
                            q2sb[g][h][:, L2P * bg:L2P * (bg + 1)],
                            start=(h == 0), stop=False)
                    nc.tensor.matmul(sc2[0:qn, 0:H * L2P], ones[0:1, 0:qn],
                                     mb2[0:1, H * L2P * b:H * L2P * (b + 1)],
                                     start=False, stop=True)
                    p2 = wk.tile([128, H * L2P], bf16, tag="p2")
                    nc.scalar.activation(p2[0:qn, :], sc2[0:qn, 0:H * L2P], Exp)
                    sums2 = wk.tile([128, H], f32, tag="sums2")
                    nc.vector.reduce_sum(
                        sums2[0:qn, :],
                        p2[0:qn, :].rearrange("p (h k) -> p h k", h=H),
                        axis=X)
                    r2 = wk.tile([128, H], f32, tag="r2")
                    nc.vector.reciprocal(r2[0:qn, :], sums2[0:qn, :])
                    p2w = wk.tile([128, H * L2P], bf16, tag="p2w")
                    nc.vector.tensor_mul(
                        p2w[0:qn, :].rearrange("p (h k) -> p h k", h=H),
                        p2[0:qn, :].rearrange("p (h k) -> p h k", h=H),
                        r2[0:qn, :].broadcast_to((qn, H, L2P)))
                    t1 = wk.tile([128, 4 * L2P], bf16, tag="t1")
                    nc.vector.tensor_add(t1[0:qn, :], p2w[0:qn, 0:4 * L2P],
                                         p2w[0:qn, 4 * L2P:8 * L2P])
                    t2 = wk.tile([128, 2 * L2P], bf16, tag="t2")
                    nc.vector.tensor_add(t2[0:qn, :], t1[0:qn, 0:2 * L2P],
                                         t1[0:qn, 2 * L2P:4 * L2P])
                    a2s = wk.tile([128, L2P], bf16, tag="a2s")
                    nc.vector.tensor_add(a2s[0:qn, :], t2[0:qn, 0:L2P],
                                         t2[0:qn, L2P:2 * L2P])
                    a2ps = smallp.tile([L2P, 1024], bf16, tag="small")
                    nc.tensor.transpose(a2ps[0:L2P, 0:qn], a2s[0:qn, 0:L2P],
                                        ident[0:qn, 0:qn])
                    nc.scalar.copy(a2t[:, 128 * c:128 * c + qn], a2ps[0:L2P, 0:qn])
                    # out2 chunk immediately (overlaps the other chunk's softmax)
                    orow = 0 if c == 0 else 128
                    for nn2 in range(2):
                        if p == pairs_limit - 1:
                            o2ps = projp.tile([128, 512], f32, tag="proj")
                        else:
                            o2ps = outp.tile([128, 512], f32, tag="outps")
                        nc.tensor.matmul(o2ps[0:qn, :],
                                         a2t[:, 128 * c:128 * c + qn],
                                         v2nb[:, 512 * nn2:512 * (nn2 + 1)],
                                         start=True, stop=True)
                        o2sb = opool.tile([128, 512], f32, tag="o2sb")
                        nc.vector.tensor_copy(o2sb[0:qn, :], o2ps[0:qn, :])
                        nc.sync.dma_start(d_o2[b, orow:orow + qn, 512 * nn2:512 * (nn2 + 1)],
                                          o2sb[0:qn, :])

    nc.compile()
    return nc


def _host_prep(value1, value2, mask1, mask2, W1, W2, none1, none2):
    """Build per-core input maps (numpy only)."""
    value1 = np.asarray(value1, dtype=np.float32)
    value2 = np.asarray(value2, dtype=np.float32)
    mask1 = np.asarray(mask1, dtype=np.float32)
    mask2 = np.asarray(mask2, dtype=np.float32)
    W1 = np.asarray(W1, dtype=np.float32)
    W2 = np.asarray(W2, dtype=np.float32)
    none1 = np.asarray(none1, dtype=np.float32)
    none2 = np.asarray(none2, dtype=np.float32)

    # k-major weight tiles: w1t[j][p, 128*d + c] = W1[128*j + c, 128*d + p]
    w1t = np.ascontiguousarray(
        W1.T.reshape(D1 // 128, 128, H, 128).transpose(2, 1, 0, 3)
        .reshape(H, 128, D1)).astype(BF16)
    w2t = np.ascontiguousarray(
        (W2 / np.sqrt(DK)).T.reshape(D2 // 128, 128, H, 128).transpose(2, 1, 0, 3)
        .reshape(H, 128, D2)).astype(BF16)
    ecst = np.zeros((128, L2), np.float32)
    for hh in range(4):
        # query q (3..32) sits at row 32*hh + (q - 1); output col is q - 3
        ecst[32 * hh + np.arange(L2) + 2, np.arange(L2)] = 1.0 / H
    ecst = ecst.astype(BF16)
    onesv = np.ones((1, 128), BF16)
    ident = np.eye(128, dtype=np.float32).astype(BF16)

    shared = {"w1t": w1t, "w2t": w2t, "ecst": ecst, "onesv": onesv, "ident": ident}

    in_maps = []
    for c in range(NCORES):
        sl = slice(BC * c, BC * (c + 1))
        v1 = np.concatenate(
            [np.broadcast_to(none1[None], (BC, NN, D1)), value1[sl]], axis=1)  # [16,199,2048]
        v2 = np.concatenate(
            [np.broadcast_to(none2[None], (BC, NN, D2)), value2[sl]], axis=1)  # [16,33,1024]
        v1n = v1.astype(BF16)
        v1t = (v1.transpose(0, 2, 1)                                   # [16,2048,199]
               .reshape(PAIRS, 2, D1, L1P).transpose(0, 2, 1, 3)
               .reshape(PAIRS, D1, 2 * L1P)).astype(BF16)
        v2n = (v2 / H).astype(BF16)
        v2tg = (v2.transpose(0, 2, 1)                                  # [16,1024,33]
                .reshape(2, GB, D2, L2P).transpose(0, 2, 1, 3)
                .reshape(2, D2, GB * L2P)).astype(BF16)
        m1f = np.concatenate([np.ones((BC, NN), np.float32), mask1[sl]], axis=1)
        m2f = np.concatenate([np.ones((BC, NN), np.float32), mask2[sl]], axis=1)
        mb1 = ((m1f - 1.0) * 30000.0).astype(BF16).reshape(1, BC * L1P)
        mb2 = np.tile(((m2f - 1.0) * 30000.0), (1, H)).astype(BF16).reshape(1, BC * H * L2P)
        in_maps.append(dict(shared, v1t=np.ascontiguousarray(v1t),
                            v1n=np.ascontiguousarray(v1n),
                            v2tg=np.ascontiguousarray(v2tg),
                            v2n=np.ascontiguousarray(v2n),
                            mb1=mb1, mb2=mb2))
    return in_maps


def _build_callable(nc):
    """jit-once PJRT callable over 8 cores (mirrors bass2jax.run_bass_via_pjrt)."""
    import jax
    from jax.sharding import Mesh, PartitionSpec
    from jax.experimental.shard_map import shard_map
    from concourse import bass2jax, mybir

    bass2jax.install_neuronx_cc_hook()
    partition_name = nc.partition_id_tensor.name if nc.partition_id_tensor else None
    in_names, out_names, out_avals, zero_outs = [], [], [], []
    for alloc in nc.m.functions[0].allocations:
        if not isinstance(alloc, mybir.MemoryLocationSet):
            continue
        name = alloc.memorylocations[0].name
        if alloc.kind == "ExternalInput":
            if name != partition_name:
                in_names.append(name)
        elif alloc.kind == "ExternalOutput":
            out_names.append(name)
            shape = tuple(alloc.tensor_shape)
            dtype = mybir.dt.np(alloc.dtype)
            out_avals.append(jax.core.ShapedArray(shape, dtype))
            zero_outs.append(np.zeros(shape, dtype))
    all_in_names = list(in_names) + list(out_names)
    if partition_name is not None:
        all_in_names.append(partition_name)

    def _body(*args):
        operands = list(args)
        if partition_name is not None:
            operands.append(bass2jax.partition_id_tensor())
        outs = bass2jax._bass_exec_p.bind(
            *operands,
            out_avals=tuple(out_avals),
            in_names=tuple(all_in_names),
            out_names=tuple(out_names),
            lowering_input_output_aliases=(),
            sim_require_finite=True,
            sim_require_nnan=True,
            nc=nc,
        )
        return tuple(outs)

    devices = jax.devices()[:NCORES]
    mesh = Mesh(np.asarray(devices), ("core",))
    n = len(in_names) + len(out_avals)
    fn = jax.jit(
        shard_map(_body, mesh=mesh, in_specs=(PartitionSpec("core"),) * n,
                  out_specs=(PartitionSpec("core"),) * len(out_names),
                  check_rep=False),
        keep_unused=True)
    return fn, in_names, out_names, zero_outs


def kernel(value1, value2, mask1, mask2, W1, W2, none1, none2):
    in_maps = _host_prep(value1, value2, mask1, mask2, W1, W2, none1, none2)

    if "nc" not in _CACHE:
        _CACHE["nc"] = _build_nc()
    nc = _CACHE["nc"]

    try:
        if "fn" not in _CACHE:
            _CACHE["fn"] = _build_callable(nc)
        fn, in_names, out_names, zero_outs = _CACHE["fn"]
        cat_in = [np.concatenate([np.asarray(m[name]) for m in in_maps], axis=0)
                  for name in in_names]
        cat_zero = [np.zeros((NCORES * z.shape[0], *z.shape[1:]), z.dtype)
                    for z in zero_outs]
        outs = fn(*cat_in, *cat_zero)
        o1 = np.asarray(outs[out_names.index("o1")])
        o2 = np.asarray(outs[out_names.index("o2")])
        return (np.ascontiguousarray(o1.reshape(B, L2, D1).astype(np.float32)),
                np.ascontiguousarray(o2.reshape(B, L1, D2).astype(np.float32)))
    except Exception:
        from concourse.bass_utils import run_bass_kernel_spmd
        res = run_bass_kernel_spmd(nc, in_maps, core_ids=list(range(NCORES)))
        out1 = np.empty((B, L2, D1), np.float32)
        out2 = np.empty((B, L1, D2), np.float32)
        for c in range(NCORES):
            out1[BC * c:BC * (c + 1)] = res.results[c]["o1"]
            out2[BC * c:BC * (c + 1)] = res.results[c]["o2"]
        return out1, out2

